# revision 7
# baseline (speedup 1.0000x reference)
"""Trainium2 Bass kernel: single-step 2-layer GRU block (batch=1), tensor-parallel
across 8 NeuronCores.

Sharding: each core owns a 256-wide slice of the 2048-wide hidden dim for both
GRU layers (row-sharding of the 3H gate weights), computes its slice of h0/h1,
and a partial of the 64-wide output projection.  The only device collective is
one 8-rank AllGather of h0 (needed as the layer-1 input); the final 64-float
output partials are summed on host.

Matvec strategy: the *vector* is the PE stationary operand ([K=128, 1] chunks)
and the host-pre-transposed weight chunks [128, 768] stream as the moving
operand (bitcast to float32r / tf32 for full-rate streaming; PSUM accumulates
in fp32).  This keeps TensorE comfortably faster than the HBM DMA of the
~21 MB/core weight shards, which is the roofline for this memory-bound problem.
"""

import numpy as np

from concourse import bacc, mybir, tile
from concourse.bass_utils import run_bass_kernel_spmd

M = 8                      # cores
IO, LIN, MEM = 64, 1024, 2048
S = MEM // M               # 256 : h-slice per core
R = 3 * S                  # 768 : gate rows per core (r,z,n slices concatenated)
KI0 = LIN // 128           # 8  : K-chunks for w_ih0 (input dim LIN)
KH = MEM // 128            # 16 : K-chunks for MEM-dim contractions
F32 = mybir.dt.float32
F32R = mybir.dt.float32r

_CACHE = {}


def _build():
    nc = bacc.Bacc("TRN2", target_bir_lowering=False, debug=False, num_devices=M)

    def inp(name, shape, dt=F32):
        return nc.dram_tensor(name, list(shape), dt, kind="ExternalInput")

    def outp(name, shape):
        return nc.dram_tensor(name, list(shape), F32, kind="ExternalOutput")

    d_x = inp("x_row", (1, IO))
    d_wproc = inp("wproc_r", (128, KI0, IO))
    d_bproc = inp("bproc_c", (128, KI0))
    d_m0c = inp("m0_col", (128, KH), F32R)
    d_m1c = inp("m1_col", (128, KH), F32R)
    d_m0v = inp("m0_vec", (1, S))
    d_m1v = inp("m1_vec", (1, S))
    d_res = inp("res_vec", (1, S))
    d_bi0 = inp("bi0", (1, R))
    d_bh0 = inp("bh0", (1, R))
    d_bi1 = inp("bi1", (1, R))
    d_bh1 = inp("bh1", (1, R))
    d_wi0 = inp("wih0t", (KI0, 128, R), F32R)
    d_wh0 = inp("whh0t", (KH, 128, R), F32R)
    d_wh1 = inp("whh1t", (KH, 128, R), F32R)
    d_wi1 = inp("wih1t", (KH, 128, R), F32R)
    d_wout = inp("woutn", (IO, S))

    o_h0 = outp("h0_out", (1, S))
    o_h1 = outp("h1_out", (1, S))
    o_nres = outp("nres_out", (1, S))
    o_op = outp("opart_out", (IO, 1))

    add = mybir.AluOpType.add
    sub = mybir.AluOpType.subtract
    mult = mybir.AluOpType.mult
    SIG = mybir.ActivationFunctionType.Sigmoid
    TANH = mybir.ActivationFunctionType.Tanh

    with tile.TileContext(nc) as tc:
        with (
            tc.tile_pool(name="stat", bufs=1) as sp,
            tc.tile_pool(name="wmov", bufs=1) as wp,
            tc.tile_pool(name="tmp", bufs=2) as vp,
            tc.tile_pool(name="ps", bufs=1, space="PSUM") as pp,
            tc.tile_pool(name="dram", bufs=1, space="DRAM") as dp,
        ):
            dma = nc.sync.dma_start

            # ---------- static small loads ----------
            x_sb = sp.tile([1, IO], F32, name="x_sb")
            dma(x_sb[:], d_x[:, :])
            wpr_sb = sp.tile([128, KI0, IO], F32, name="wpr_sb")
            dma(wpr_sb[:], d_wproc[:, :, :])
            bpr_sb = sp.tile([128, KI0], F32, name="bpr_sb")
            dma(bpr_sb[:], d_bproc[:, :])
            m0c_sb = sp.tile([128, KH], F32R, name="m0c_sb")
            dma(m0c_sb[:], d_m0c[:, :])
            m1c_sb = sp.tile([128, KH], F32R, name="m1c_sb")
            dma(m1c_sb[:], d_m1c[:, :])
            m0v_sb = sp.tile([1, S], F32, name="m0v_sb")
            dma(m0v_sb[:], d_m0v[:, :])
            m1v_sb = sp.tile([1, S], F32, name="m1v_sb")
            dma(m1v_sb[:], d_m1v[:, :])
            res_sb = sp.tile([1, S], F32, name="res_sb")
            dma(res_sb[:], d_res[:, :])
            bi0_sb = sp.tile([1, R], F32, name="bi0_sb")
            dma(bi0_sb[:], d_bi0[:, :])
            bh0_sb = sp.tile([1, R], F32, name="bh0_sb")
            dma(bh0_sb[:], d_bh0[:, :])
            bi1_sb = sp.tile([1, R], F32, name="bi1_sb")
            dma(bi1_sb[:], d_bi1[:, :])
            bh1_sb = sp.tile([1, R], F32, name="bh1_sb")
            dma(bh1_sb[:], d_bh1[:, :])
            wout_sb = sp.tile([IO, S], F32, name="wout_sb")
            dma(wout_sb[:], d_wout[:, :])

            # ---------- p = W_proc @ x + b_proc, laid out [128, 8] ----------
            # broadcast x to all 128 partitions via a rank-1 PE outer product
            ones_sb = sp.tile([1, 128], F32, name="ones_sb")
            nc.vector.memset(ones_sb[:], 1.0)
            xb_ps = pp.tile([128, IO], F32, name="xb_ps", tag="xb", bufs=1)
            nc.tensor.matmul(
                xb_ps[:], lhsT=ones_sb[:], rhs=x_sb[:], start=True, stop=True
            )
            xb_sb = sp.tile([128, IO], F32, name="xb_sb")
            nc.vector.tensor_copy(xb_sb[:], xb_ps[:])
            praw_sb = sp.tile([128, KI0], F32, name="praw_sb")
            for j in range(KI0):
                prod = vp.tile([128, IO], F32, name=f"pprod{j}", tag="pprod", bufs=2)
                nc.vector.tensor_tensor(prod[:], wpr_sb[:, j, :], xb_sb[:], mult)
                nc.vector.tensor_reduce(
                    praw_sb[:, j : j + 1], prod[:], mybir.AxisListType.X, add
                )
            p_sb = sp.tile([128, KI0], F32R, name="p_sb")
            nc.vector.tensor_tensor(p_sb[:], praw_sb[:], bpr_sb[:], add)

            # ---------- gate matvec helper ----------
            wcount = [0]

            def gates(name, nk, w_dram, stat_sb, ps_tag, ps_bufs):
                """psum[1, R] += sum_k stat[:,k].T @ w_chunk_k  (tf32 stream)."""
                g_ps = pp.tile([1, R], F32, name=f"g_{name}", tag=ps_tag, bufs=ps_bufs)
                for k in range(nk):
                    i = wcount[0]
                    wcount[0] += 1
                    wt = wp.tile([128, R], F32R, name=f"w_{name}_{k}", tag="wmov", bufs=24)
                    dma(wt[:], w_dram[k, :, :])
                    lt = stat_sb[:, k : k + 1]
                    nc.tensor.matmul(
                        g_ps[0:1, 0:512],
                        lhsT=lt,
                        rhs=wt[:, 0:512],
                        start=(k == 0),
                        stop=(k == nk - 1),
                    )
                    nc.tensor.matmul(
                        g_ps[0:1, 512:R],
                        lhsT=lt,
                        rhs=wt[:, 512:R],
                        start=(k == 0),
                        stop=(k == nk - 1),
                    )
                return g_ps

            def gate_math(name, gi_ps, gh_ps, bi_sb, bh_sb, mv_sb, h_sb):
                """h = (1-z)*n + z*m with r,z,n from gi/gh psum, all [1, *]."""
                gib = vp.tile([1, R], F32, name=f"gib_{name}", tag="gib", bufs=2)
                nc.vector.tensor_tensor(gib[:], gi_ps[:], bi_sb[:], add)
                ghb = vp.tile([1, R], F32, name=f"ghb_{name}", tag="ghb", bufs=2)
                nc.vector.tensor_tensor(ghb[:], gh_ps[:], bh_sb[:], add)
                srz = vp.tile([1, 2 * S], F32, name=f"srz_{name}", tag="srz", bufs=2)
                nc.vector.tensor_tensor(
                    srz[:], gib[:, 0 : 2 * S], ghb[:, 0 : 2 * S], add
                )
                rz = vp.tile([1, 2 * S], F32, name=f"rz_{name}", tag="rz", bufs=2)
                nc.scalar.activation(rz[:], srz[:], SIG)
                t = vp.tile([1, S], F32, name=f"t_{name}", tag="t", bufs=2)
                nc.vector.tensor_tensor(t[:], rz[:, 0:S], ghb[:, 2 * S : R], mult)
                u = vp.tile([1, S], F32, name=f"u_{name}", tag="u", bufs=2)
                nc.vector.tensor_tensor(u[:], gib[:, 2 * S : R], t[:], add)
                n = vp.tile([1, S], F32, name=f"n_{name}", tag="n", bufs=2)
                nc.scalar.activation(n[:], u[:], TANH)
                d = vp.tile([1, S], F32, name=f"d_{name}", tag="d", bufs=2)
                nc.vector.tensor_tensor(d[:], mv_sb[:], n[:], sub)
                e = vp.tile([1, S], F32, name=f"e_{name}", tag="e", bufs=2)
                nc.vector.tensor_tensor(e[:], rz[:, S : 2 * S], d[:], mult)
                nc.vector.tensor_tensor(h_sb[:], n[:], e[:], add)

            # ---------- layer 0 ----------
            gi0_ps = gates("gi0", KI0, d_wi0, p_sb, "gi", 1)
            gh0_ps = gates("gh0", KH, d_wh0, m0c_sb, "gh", 2)
            h0v_sb = sp.tile([1, S], F32, name="h0v_sb")
            gate_math("l0", gi0_ps, gh0_ps, bi0_sb, bh0_sb, m0v_sb, h0v_sb)
            dma(o_h0[:, :], h0v_sb[:])

            # ---------- AllGather h0 (8 x 256 -> 2048) ----------
            ag_in = dp.tile([1, S], F32R, name="ag_in")
            ag_out = dp.tile([KH, 128], F32R, name="ag_out")
            nc.gpsimd.dma_start(ag_in[:], h0v_sb[:])  # f32 -> f32r cast (SWDGE)
            nc.gpsimd.collective_compute(
                "AllGather",
                mybir.AluOpType.bypass,
                replica_groups=[list(range(M))],
                ins=[ag_in[:, :].opt()],
                outs=[ag_out[:, :].opt()],
            )
            h0c_sb = sp.tile([128, KH], F32R, name="h0c_sb")
            dma(h0c_sb[:], ag_out[:, :].rearrange("j p -> p j"))

            # ---------- layer 1 (gh1 first: no dependence on h0) ----------
            gh1_ps = gates("gh1", KH, d_wh1, m1c_sb, "gh", 2)
            gi1_ps = gates("gi1", KH, d_wi1, h0c_sb, "gi", 1)
            h1v_sb = sp.tile([1, S], F32, name="h1v_sb")
            gate_math("l1", gi1_ps, gh1_ps, bi1_sb, bh1_sb, m1v_sb, h1v_sb)
            dma(o_h1[:, :], h1v_sb[:])

            # ---------- merged/residual outputs ----------
            mrg_sb = sp.tile([1, S], F32, name="mrg_sb")
            nc.vector.tensor_tensor(mrg_sb[:], h1v_sb[:], res_sb[:], add)
            nres_sb = sp.tile([1, S], F32, name="nres_sb")
            nc.scalar.activation(nres_sb[:], mrg_sb[:], SIG)
            dma(o_nres[:, :], nres_sb[:])

            # ---------- output-projection partial: o = W_out[:, sl] @ merged ----------
            mb_ps = pp.tile([IO, S], F32, name="mb_ps", tag="xb", bufs=1)
            nc.tensor.matmul(
                mb_ps[:], lhsT=ones_sb[:, 0:IO], rhs=mrg_sb[:], start=True, stop=True
            )
            oprod = sp.tile([IO, S], F32, name="oprod")
            o_sb = sp.tile([IO, 1], F32, name="o_sb")
            nc.vector.tensor_tensor(oprod[:], wout_sb[:], mb_ps[:], mult)
            nc.vector.tensor_reduce(o_sb[:], oprod[:], mybir.AxisListType.X, add)
            dma(o_op[:, :], o_sb[:])

    nc.compile()
    return nc


def _shards(inputs):
    """Build the 8 per-core input maps from the full-size numpy inputs."""
    x = np.asarray(inputs["x"], np.float32)
    residual = np.asarray(inputs["residual"], np.float32)
    memory = np.asarray(inputs["memory"], np.float32)
    W_proc = np.asarray(inputs["W_proc"], np.float32)
    b_proc = np.asarray(inputs["b_proc"], np.float32)
    W_out = np.asarray(inputs["W_out"], np.float32)

    # shared (replicated) tensors
    x_row = np.ascontiguousarray(x.reshape(1, IO))
    wproc_r = np.ascontiguousarray(W_proc.reshape(KI0, 128, IO).transpose(1, 0, 2))
    bproc_c = np.ascontiguousarray(b_proc.reshape(KI0, 128).T)
    m0 = memory[0, 0]
    m1 = memory[1, 0]
    m0_col = np.ascontiguousarray(m0.reshape(KH, 128).T)
    m1_col = np.ascontiguousarray(m1.reshape(KH, 128).T)

    maps = []
    for c in range(M):
        sl = np.arange(S * c, S * (c + 1))
        rows = np.concatenate([sl, MEM + sl, 2 * MEM + sl])
        mp = {
            "x_row": x_row,
            "wproc_r": wproc_r,
            "bproc_c": bproc_c,
            "m0_col": m0_col,
            "m1_col": m1_col,
            "m0_vec": np.ascontiguousarray(m0[sl].reshape(1, S)),
            "m1_vec": np.ascontiguousarray(m1[sl].reshape(1, S)),
            "res_vec": np.ascontiguousarray(residual[0, 0, sl].reshape(1, S)),
            "bi0": np.ascontiguousarray(inputs["b_ih0"][rows].reshape(1, R)).astype(np.float32),
            "bh0": np.ascontiguousarray(inputs["b_hh0"][rows].reshape(1, R)).astype(np.float32),
            "bi1": np.ascontiguousarray(inputs["b_ih1"][rows].reshape(1, R)).astype(np.float32),
            "bh1": np.ascontiguousarray(inputs["b_hh1"][rows].reshape(1, R)).astype(np.float32),
            "wih0t": np.ascontiguousarray(np.asarray(inputs["w_ih0"], np.float32)[rows].T).reshape(KI0, 128, R),
            "whh0t": np.ascontiguousarray(np.asarray(inputs["w_hh0"], np.float32)[rows].T).reshape(KH, 128, R),
            "whh1t": np.ascontiguousarray(np.asarray(inputs["w_hh1"], np.float32)[rows].T).reshape(KH, 128, R),
            "wih1t": np.ascontiguousarray(np.asarray(inputs["w_ih1"], np.float32)[rows].T).reshape(KH, 128, R),
            "woutn": np.ascontiguousarray(W_out[:, sl]),
        }
        maps.append(mp)
    return maps


def _run(inputs, **kwargs):
    if "nc" not in _CACHE:
        _CACHE["nc"] = _build()
    nc = _CACHE["nc"]
    in_maps = _shards(inputs)
    res = run_bass_kernel_spmd(nc, in_maps, core_ids=list(range(M)), **kwargs)
    return res


def _assemble(results, inputs):
    b_out = np.asarray(inputs["b_out"], np.float32)
    h0 = np.concatenate([results[c]["h0_out"].ravel() for c in range(M)])
    h1 = np.concatenate([results[c]["h1_out"].ravel() for c in range(M)])
    nres = np.concatenate([results[c]["nres_out"].ravel() for c in range(M)])
    opart = np.sum([results[c]["opart_out"].ravel() for c in range(M)], axis=0)
    output = (opart + b_out).astype(np.float32).reshape(1, 1, IO)
    new_residual = nres.astype(np.float32).reshape(1, 1, MEM)
    new_memory = np.stack([h0, h1]).astype(np.float32).reshape(2, 1, MEM)
    return output, new_residual, new_memory


def kernel(**inputs):
    res = _run(inputs)
    return _assemble(res.results, inputs)


# revision 8
# speedup vs baseline: 1.1033x; 1.1033x over previous
"""Trainium2 Bass kernel: single-step 2-layer GRU block (batch=1), tensor-parallel
across 8 NeuronCores.

Sharding: each core owns a 256-wide slice of the 2048-wide hidden dim for both
GRU layers (row-sharding of the 3H gate weights), computes its slice of h0/h1,
and a partial of the 64-wide output projection.  The only device collective is
one 8-rank AllGather of h0 (needed as the layer-1 input); the final 64-float
output partials are summed on host.

Matvec strategy: the *vector* is the PE stationary operand ([K=128, 1] chunks)
and host-pre-transposed fp16 weight chunks [128, 768] stream as the moving
operand (fp16 moving = 1 col/cycle on the PE; PSUM accumulates in fp32).
Weights are cast to fp16 on the host: same 10-bit mantissa class as tf32,
half the HBM/DMA bytes — this problem is memory-bound on the weight stream.
"""

import numpy as np

from concourse import bacc, mybir, tile
from concourse.bass_utils import run_bass_kernel_spmd

M = 8                      # cores
IO, LIN, MEM = 64, 1024, 2048
S = MEM // M               # 256 : h-slice per core
R = 3 * S                  # 768 : gate rows per core (r,z,n slices concatenated)
KI0 = LIN // 128           # 8  : K-chunks for w_ih0 (input dim LIN)
KH = MEM // 128            # 16 : K-chunks for MEM-dim contractions
GB = 4                     # K-chunks per DMA group
F32 = mybir.dt.float32
F16 = mybir.dt.float16

_CACHE = {}


def _build():
    nc = bacc.Bacc("TRN2", target_bir_lowering=False, debug=False, num_devices=M)

    def inp(name, shape, dt=F32):
        return nc.dram_tensor(name, list(shape), dt, kind="ExternalInput")

    def outp(name, shape):
        return nc.dram_tensor(name, list(shape), F32, kind="ExternalOutput")

    d_x = inp("x_row", (1, IO))
    d_wproc = inp("wproc_r", (128, KI0, IO))
    d_bproc = inp("bproc_c", (128, KI0))
    d_m0c = inp("m0_col", (128, KH), F16)
    d_m1c = inp("m1_col", (128, KH), F16)
    d_m0v = inp("m0_vec", (1, S))
    d_m1v = inp("m1_vec", (1, S))
    d_res = inp("res_vec", (1, S))
    d_bi0 = inp("bi0", (1, R))
    d_bh0 = inp("bh0", (1, R))
    d_bi1 = inp("bi1", (1, R))
    d_bh1 = inp("bh1", (1, R))
    d_wi0 = inp("wih0t", (KI0 // GB, GB, 128, R), F16)
    d_wh0 = inp("whh0t", (KH // GB, GB, 128, R), F16)
    d_wh1 = inp("whh1t", (KH // GB, GB, 128, R), F16)
    d_wi1 = inp("wih1t", (KH // GB, GB, 128, R), F16)
    d_wout = inp("woutn", (IO, S))

    o_h0 = outp("h0_out", (1, S))
    o_h1 = outp("h1_out", (1, S))
    o_nres = outp("nres_out", (1, S))
    o_op = outp("opart_out", (IO, 1))

    add = mybir.AluOpType.add
    sub = mybir.AluOpType.subtract
    mult = mybir.AluOpType.mult
    SIG = mybir.ActivationFunctionType.Sigmoid
    TANH = mybir.ActivationFunctionType.Tanh

    with tile.TileContext(nc) as tc:
        with (
            tc.tile_pool(name="stat", bufs=1) as sp,
            tc.tile_pool(name="wmov", bufs=1) as wp,
            tc.tile_pool(name="tmp", bufs=2) as vp,
            tc.tile_pool(name="ps", bufs=1, space="PSUM") as pp,
            tc.tile_pool(name="dram", bufs=1, space="DRAM") as dp,
        ):
            dma = nc.sync.dma_start

            # ---------- static small loads ----------
            x_sb = sp.tile([1, IO], F32, name="x_sb")
            dma(x_sb[:], d_x[:, :])
            wpr_sb = sp.tile([128, KI0, IO], F32, name="wpr_sb")
            dma(wpr_sb[:], d_wproc[:, :, :])
            bpr_sb = sp.tile([128, KI0], F32, name="bpr_sb")
            dma(bpr_sb[:], d_bproc[:, :])
            m0c_sb = sp.tile([128, KH], F16, name="m0c_sb")
            dma(m0c_sb[:], d_m0c[:, :])
            m1c_sb = sp.tile([128, KH], F16, name="m1c_sb")
            dma(m1c_sb[:], d_m1c[:, :])
            m0v_sb = sp.tile([1, S], F32, name="m0v_sb")
            dma(m0v_sb[:], d_m0v[:, :])
            m1v_sb = sp.tile([1, S], F32, name="m1v_sb")
            dma(m1v_sb[:], d_m1v[:, :])
            res_sb = sp.tile([1, S], F32, name="res_sb")
            dma(res_sb[:], d_res[:, :])
            bi0_sb = sp.tile([1, R], F32, name="bi0_sb")
            dma(bi0_sb[:], d_bi0[:, :])
            bh0_sb = sp.tile([1, R], F32, name="bh0_sb")
            dma(bh0_sb[:], d_bh0[:, :])
            bi1_sb = sp.tile([1, R], F32, name="bi1_sb")
            dma(bi1_sb[:], d_bi1[:, :])
            bh1_sb = sp.tile([1, R], F32, name="bh1_sb")
            dma(bh1_sb[:], d_bh1[:, :])
            wout_sb = sp.tile([IO, S], F32, name="wout_sb")
            dma(wout_sb[:], d_wout[:, :])

            # ---------- p = W_proc @ x + b_proc, laid out [128, 8] ----------
            # broadcast x to all 128 partitions via a rank-1 PE outer product
            ones_sb = sp.tile([1, 128], F32, name="ones_sb")
            nc.vector.memset(ones_sb[:], 1.0)
            xb_ps = pp.tile([128, IO], F32, name="xb_ps", tag="xb", bufs=1)
            nc.tensor.matmul(
                xb_ps[:], lhsT=ones_sb[:], rhs=x_sb[:], start=True, stop=True
            )
            xb_sb = sp.tile([128, IO], F32, name="xb_sb")
            nc.vector.tensor_copy(xb_sb[:], xb_ps[:])
            praw_sb = sp.tile([128, KI0], F32, name="praw_sb")
            for j in range(KI0):
                prod = vp.tile([128, IO], F32, name=f"pprod{j}", tag="pprod", bufs=2)
                nc.vector.tensor_tensor(prod[:], wpr_sb[:, j, :], xb_sb[:], mult)
                nc.vector.tensor_reduce(
                    praw_sb[:, j : j + 1], prod[:], mybir.AxisListType.X, add
                )
            p_sb = sp.tile([128, KI0], F16, name="p_sb")
            nc.vector.tensor_tensor(p_sb[:], praw_sb[:], bpr_sb[:], add)

            # ---------- gate matvec helper ----------
            def gates(name, nk, w_dram, stat_sb, ps_tag, ps_bufs):
                """psum[1, R] += sum_k stat[:,k].T @ w_chunk_k  (fp16 stream)."""
                g_ps = pp.tile([1, R], F32, name=f"g_{name}", tag=ps_tag, bufs=ps_bufs)
                for g in range(nk // GB):
                    wt = wp.tile(
                        [128, GB, R], F16, name=f"w_{name}_{g}", tag=f"w_{name}_{g}"
                    )
                    dma(wt[:], w_dram[g].rearrange("c p r -> p c r"))
                    for j in range(GB):
                        k = g * GB + j
                        lt = stat_sb[:, k : k + 1]
                        nc.tensor.matmul(
                            g_ps[0:1, 0:512],
                            lhsT=lt,
                            rhs=wt[:, j, 0:512],
                            start=(k == 0),
                            stop=(k == nk - 1),
                        )
                        nc.tensor.matmul(
                            g_ps[0:1, 512:R],
                            lhsT=lt,
                            rhs=wt[:, j, 512:R],
                            start=(k == 0),
                            stop=(k == nk - 1),
                        )
                return g_ps

            def gate_math(name, gi_ps, gh_ps, bi_sb, bh_sb, mv_sb, h_sb):
                """h = (1-z)*n + z*m with r,z,n from gi/gh psum, all [1, *]."""
                gib = vp.tile([1, R], F32, name=f"gib_{name}", tag="gib", bufs=2)
                nc.vector.tensor_tensor(gib[:], gi_ps[:], bi_sb[:], add)
                ghb = vp.tile([1, R], F32, name=f"ghb_{name}", tag="ghb", bufs=2)
                nc.vector.tensor_tensor(ghb[:], gh_ps[:], bh_sb[:], add)
                srz = vp.tile([1, 2 * S], F32, name=f"srz_{name}", tag="srz", bufs=2)
                nc.vector.tensor_tensor(
                    srz[:], gib[:, 0 : 2 * S], ghb[:, 0 : 2 * S], add
                )
                rz = vp.tile([1, 2 * S], F32, name=f"rz_{name}", tag="rz", bufs=2)
                nc.scalar.activation(rz[:], srz[:], SIG)
                t = vp.tile([1, S], F32, name=f"t_{name}", tag="t", bufs=2)
                nc.vector.tensor_tensor(t[:], rz[:, 0:S], ghb[:, 2 * S : R], mult)
                u = vp.tile([1, S], F32, name=f"u_{name}", tag="u", bufs=2)
                nc.vector.tensor_tensor(u[:], gib[:, 2 * S : R], t[:], add)
                n = vp.tile([1, S], F32, name=f"n_{name}", tag="n", bufs=2)
                nc.scalar.activation(n[:], u[:], TANH)
                d = vp.tile([1, S], F32, name=f"d_{name}", tag="d", bufs=2)
                nc.vector.tensor_tensor(d[:], mv_sb[:], n[:], sub)
                e = vp.tile([1, S], F32, name=f"e_{name}", tag="e", bufs=2)
                nc.vector.tensor_tensor(e[:], rz[:, S : 2 * S], d[:], mult)
                nc.vector.tensor_tensor(h_sb[:], n[:], e[:], add)

            # ---------- layer 0 ----------
            gi0_ps = gates("gi0", KI0, d_wi0, p_sb, "gi", 1)
            gh0_ps = gates("gh0", KH, d_wh0, m0c_sb, "gh", 2)
            h0v_sb = sp.tile([1, S], F32, name="h0v_sb")
            gate_math("l0", gi0_ps, gh0_ps, bi0_sb, bh0_sb, m0v_sb, h0v_sb)
            nc.gpsimd.dma_start(o_h0[:, :], h0v_sb[:])

            # ---------- AllGather h0 (8 x 256 -> 2048), fp16 on the wire ----------
            ag_in = dp.tile([1, S], F16, name="ag_in")
            ag_out = dp.tile([KH, 128], F16, name="ag_out")
            nc.gpsimd.dma_start(ag_in[:], h0v_sb[:])  # f32 -> f16 cast (SWDGE)
            nc.gpsimd.collective_compute(
                "AllGather",
                mybir.AluOpType.bypass,
                replica_groups=[list(range(M))],
                ins=[ag_in[:, :].opt()],
                outs=[ag_out[:, :].opt()],
            )
            h0c_sb = sp.tile([128, KH], F16, name="h0c_sb")
            nc.gpsimd.dma_start(h0c_sb[:], ag_out[:, :].rearrange("j p -> p j"))

            # ---------- layer 1 (gh1 first: no dependence on h0) ----------
            gh1_ps = gates("gh1", KH, d_wh1, m1c_sb, "gh", 2)
            gi1_ps = gates("gi1", KH, d_wi1, h0c_sb, "gi", 1)
            h1v_sb = sp.tile([1, S], F32, name="h1v_sb")
            gate_math("l1", gi1_ps, gh1_ps, bi1_sb, bh1_sb, m1v_sb, h1v_sb)
            nc.gpsimd.dma_start(o_h1[:, :], h1v_sb[:])

            # ---------- merged/residual outputs ----------
            mrg_sb = sp.tile([1, S], F32, name="mrg_sb")
            nc.vector.tensor_tensor(mrg_sb[:], h1v_sb[:], res_sb[:], add)
            nres_sb = sp.tile([1, S], F32, name="nres_sb")
            nc.scalar.activation(nres_sb[:], mrg_sb[:], SIG)
            nc.gpsimd.dma_start(o_nres[:, :], nres_sb[:])

            # ---------- output-projection partial: o = W_out[:, sl] @ merged ----------
            mb_ps = pp.tile([IO, S], F32, name="mb_ps", tag="xb", bufs=1)
            nc.tensor.matmul(
                mb_ps[:], lhsT=ones_sb[:, 0:IO], rhs=mrg_sb[:], start=True, stop=True
            )
            oprod = sp.tile([IO, S], F32, name="oprod")
            o_sb = sp.tile([IO, 1], F32, name="o_sb")
            nc.vector.tensor_tensor(oprod[:], wout_sb[:], mb_ps[:], mult)
            nc.vector.tensor_reduce(o_sb[:], oprod[:], mybir.AxisListType.X, add)
            nc.gpsimd.dma_start(o_op[:, :], o_sb[:])

    nc.compile()
    return nc


def _shards(inputs):
    """Build the 8 per-core input maps from the full-size numpy inputs."""
    x = np.asarray(inputs["x"], np.float32)
    residual = np.asarray(inputs["residual"], np.float32)
    memory = np.asarray(inputs["memory"], np.float32)
    W_proc = np.asarray(inputs["W_proc"], np.float32)
    b_proc = np.asarray(inputs["b_proc"], np.float32)
    W_out = np.asarray(inputs["W_out"], np.float32)

    # shared (replicated) tensors
    x_row = np.ascontiguousarray(x.reshape(1, IO))
    wproc_r = np.ascontiguousarray(W_proc.reshape(KI0, 128, IO).transpose(1, 0, 2))
    bproc_c = np.ascontiguousarray(b_proc.reshape(KI0, 128).T)
    m0 = memory[0, 0]
    m1 = memory[1, 0]
    m0_col = np.ascontiguousarray(m0.reshape(KH, 128).T.astype(np.float16))
    m1_col = np.ascontiguousarray(m1.reshape(KH, 128).T.astype(np.float16))

    def wt(w, rows, nk):
        w16 = np.asarray(w, np.float32)[rows].T.astype(np.float16)
        return np.ascontiguousarray(w16).reshape(nk // GB, GB, 128, R)

    maps = []
    for c in range(M):
        sl = np.arange(S * c, S * (c + 1))
        rows = np.concatenate([sl, MEM + sl, 2 * MEM + sl])
        mp = {
            "x_row": x_row,
            "wproc_r": wproc_r,
            "bproc_c": bproc_c,
            "m0_col": m0_col,
            "m1_col": m1_col,
            "m0_vec": np.ascontiguousarray(m0[sl].reshape(1, S)),
            "m1_vec": np.ascontiguousarray(m1[sl].reshape(1, S)),
            "res_vec": np.ascontiguousarray(residual[0, 0, sl].reshape(1, S)),
            "bi0": np.ascontiguousarray(inputs["b_ih0"][rows].reshape(1, R)).astype(np.float32),
            "bh0": np.ascontiguousarray(inputs["b_hh0"][rows].reshape(1, R)).astype(np.float32),
            "bi1": np.ascontiguousarray(inputs["b_ih1"][rows].reshape(1, R)).astype(np.float32),
            "bh1": np.ascontiguousarray(inputs["b_hh1"][rows].reshape(1, R)).astype(np.float32),
            "wih0t": wt(inputs["w_ih0"], rows, KI0),
            "whh0t": wt(inputs["w_hh0"], rows, KH),
            "whh1t": wt(inputs["w_hh1"], rows, KH),
            "wih1t": wt(inputs["w_ih1"], rows, KH),
            "woutn": np.ascontiguousarray(W_out[:, sl]),
        }
        maps.append(mp)
    return maps


def _run(inputs, **kwargs):
    if "nc" not in _CACHE:
        _CACHE["nc"] = _build()
    nc = _CACHE["nc"]
    in_maps = _shards(inputs)
    res = run_bass_kernel_spmd(nc, in_maps, core_ids=list(range(M)), **kwargs)
    return res


def _assemble(results, inputs):
    b_out = np.asarray(inputs["b_out"], np.float32)
    h0 = np.concatenate([results[c]["h0_out"].ravel() for c in range(M)])
    h1 = np.concatenate([results[c]["h1_out"].ravel() for c in range(M)])
    nres = np.concatenate([results[c]["nres_out"].ravel() for c in range(M)])
    opart = np.sum([results[c]["opart_out"].ravel() for c in range(M)], axis=0)
    output = (opart + b_out).astype(np.float32).reshape(1, 1, IO)
    new_residual = nres.astype(np.float32).reshape(1, 1, MEM)
    new_memory = np.stack([h0, h1]).astype(np.float32).reshape(2, 1, MEM)
    return output, new_residual, new_memory


def kernel(**inputs):
    res = _run(inputs)
    return _assemble(res.results, inputs)


# revision 9
# speedup vs baseline: 1.2479x; 1.1310x over previous
"""Trainium2 Bass kernel: single-step 2-layer GRU block (batch=1), tensor-parallel
across 8 NeuronCores.

Sharding: each core owns a 256-wide slice of the 2048-wide hidden dim for both
GRU layers (row-sharding of the 3H gate weights), computes its slice of h0/h1,
and a partial of the 64-wide output projection.  The only device collective is
one 8-rank AllGather of h0 (needed as the layer-1 input); the final 64-float
output partials are summed on host.

Matvec strategy: the *vector* is the PE stationary operand ([K=128, 1] chunks)
and host-pre-transposed fp16 weight chunks [128, 768] stream as the moving
operand (fp16 moving = 1 col/cycle on the PE; PSUM accumulates in fp32).
Weights are cast to fp16 on the host: same 10-bit mantissa class as tf32,
half the HBM/DMA bytes — this problem is memory-bound on the weight stream.
"""

import numpy as np

from concourse import bacc, mybir, tile
from concourse.bass_utils import run_bass_kernel_spmd

M = 8                      # cores
IO, LIN, MEM = 64, 1024, 2048
S = MEM // M               # 256 : h-slice per core
R = 3 * S                  # 768 : gate rows per core (r,z,n slices concatenated)
KI0 = LIN // 128           # 8  : K-chunks for w_ih0 (input dim LIN)
KH = MEM // 128            # 16 : K-chunks for MEM-dim contractions
GB = 4                     # K-chunks per DMA group
F32 = mybir.dt.float32
F16 = mybir.dt.float16

_CACHE = {}


def _build():
    nc = bacc.Bacc("TRN2", target_bir_lowering=False, debug=False, num_devices=M)

    def inp(name, shape, dt=F32):
        return nc.dram_tensor(name, list(shape), dt, kind="ExternalInput")

    def outp(name, shape):
        return nc.dram_tensor(name, list(shape), F32, kind="ExternalOutput")

    d_x = inp("x_row", (1, IO))
    d_wproc = inp("wproc_r", (128, KI0, IO))
    d_bproc = inp("bproc_c", (128, KI0))
    d_m0c = inp("m0_col", (128, KH), F16)
    d_m1c = inp("m1_col", (128, KH), F16)
    d_m0v = inp("m0_vec", (1, S))
    d_m1v = inp("m1_vec", (1, S))
    d_res = inp("res_vec", (1, S))
    d_bi0 = inp("bi0", (1, R))
    d_bh0 = inp("bh0", (1, R))
    d_bi1 = inp("bi1", (1, R))
    d_bh1 = inp("bh1", (1, R))
    d_wi0 = inp("wih0t", (KI0 // GB, GB, 128, R), F16)
    d_wh0 = inp("whh0t", (KH // GB, GB, 128, R), F16)
    d_wh1 = inp("whh1t", (KH // GB, GB, 128, R), F16)
    d_wi1 = inp("wih1t", (KH // GB, GB, 128, R), F16)
    d_wout = inp("woutn", (IO, S))

    o_h0 = outp("h0_out", (1, S))
    o_h1 = outp("h1_out", (1, S))
    o_nres = outp("nres_out", (1, S))
    o_op = outp("opart_out", (IO, 1))

    add = mybir.AluOpType.add
    sub = mybir.AluOpType.subtract
    mult = mybir.AluOpType.mult
    SIG = mybir.ActivationFunctionType.Sigmoid
    TANH = mybir.ActivationFunctionType.Tanh

    with tile.TileContext(nc) as tc:
        with (
            tc.tile_pool(name="stat", bufs=1) as sp,
            tc.tile_pool(name="wmov", bufs=1) as wp,
            tc.tile_pool(name="tmp", bufs=2) as vp,
            tc.tile_pool(name="ps", bufs=1, space="PSUM") as pp,
            tc.tile_pool(name="dram", bufs=1, space="DRAM") as dp,
        ):
            dma = nc.sync.dma_start

            # ---------- static small loads ----------
            x_sb = sp.tile([1, IO], F32, name="x_sb")
            dma(x_sb[:], d_x[:, :])
            wpr_sb = sp.tile([128, KI0, IO], F32, name="wpr_sb")
            dma(wpr_sb[:], d_wproc[:, :, :])
            bpr_sb = sp.tile([128, KI0], F32, name="bpr_sb")
            dma(bpr_sb[:], d_bproc[:, :])
            m0c_sb = sp.tile([128, KH], F16, name="m0c_sb")
            dma(m0c_sb[:], d_m0c[:, :])
            m1c_sb = sp.tile([128, KH], F16, name="m1c_sb")
            dma(m1c_sb[:], d_m1c[:, :])
            m0v_sb = sp.tile([1, S], F32, name="m0v_sb")
            dma(m0v_sb[:], d_m0v[:, :])
            m1v_sb = sp.tile([1, S], F32, name="m1v_sb")
            dma(m1v_sb[:], d_m1v[:, :])
            res_sb = sp.tile([1, S], F32, name="res_sb")
            dma(res_sb[:], d_res[:, :])
            bi0_sb = sp.tile([1, R], F32, name="bi0_sb")
            dma(bi0_sb[:], d_bi0[:, :])
            bh0_sb = sp.tile([1, R], F32, name="bh0_sb")
            dma(bh0_sb[:], d_bh0[:, :])
            bi1_sb = sp.tile([1, R], F32, name="bi1_sb")
            dma(bi1_sb[:], d_bi1[:, :])
            bh1_sb = sp.tile([1, R], F32, name="bh1_sb")
            dma(bh1_sb[:], d_bh1[:, :])
            wout_sb = sp.tile([IO, S], F32, name="wout_sb")
            dma(wout_sb[:], d_wout[:, :])

            # ---------- p = W_proc @ x + b_proc, laid out [128, 8] ----------
            # broadcast x to all 128 partitions via a rank-1 PE outer product
            ones_sb = sp.tile([1, 128], F32, name="ones_sb")
            nc.vector.memset(ones_sb[:], 1.0)
            xb_ps = pp.tile([128, IO], F32, name="xb_ps", tag="xb", bufs=1)
            nc.tensor.matmul(
                xb_ps[:], lhsT=ones_sb[:], rhs=x_sb[:], start=True, stop=True
            )
            xb_sb = sp.tile([128, IO], F32, name="xb_sb")
            nc.vector.tensor_copy(xb_sb[:], xb_ps[:])
            praw_sb = sp.tile([128, KI0], F32, name="praw_sb")
            for j in range(KI0):
                prod = vp.tile([128, IO], F32, name=f"pprod{j}", tag="pprod", bufs=2)
                nc.vector.tensor_tensor(prod[:], wpr_sb[:, j, :], xb_sb[:], mult)
                nc.vector.tensor_reduce(
                    praw_sb[:, j : j + 1], prod[:], mybir.AxisListType.X, add
                )
            p_sb = sp.tile([128, KI0], F16, name="p_sb")
            nc.vector.tensor_tensor(p_sb[:], praw_sb[:], bpr_sb[:], add)

            # ---------- gate matvec helper ----------
            def gates(name, nk, w_dram, stat_sb, ps_tag, ps_bufs):
                """psum[1, R] += sum_k stat[:,k].T @ w_chunk_k  (fp16 stream)."""
                g_ps = pp.tile([1, R], F32, name=f"g_{name}", tag=ps_tag, bufs=ps_bufs)
                for g in range(nk // GB):
                    wt = wp.tile(
                        [128, GB, R], F16, name=f"w_{name}_{g}", tag=f"w_{name}_{g}"
                    )
                    dma(wt[:], w_dram[g].rearrange("c p r -> p c r"))
                    for j in range(GB):
                        k = g * GB + j
                        lt = stat_sb[:, k : k + 1]
                        nc.tensor.matmul(
                            g_ps[0:1, 0:512],
                            lhsT=lt,
                            rhs=wt[:, j, 0:512],
                            start=(k == 0),
                            stop=(k == nk - 1),
                        )
                        nc.tensor.matmul(
                            g_ps[0:1, 512:R],
                            lhsT=lt,
                            rhs=wt[:, j, 512:R],
                            start=(k == 0),
                            stop=(k == nk - 1),
                        )
                return g_ps

            def gate_math(name, gi_ps, gh_ps, bi_sb, bh_sb, mv_sb, h_sb):
                """h = (1-z)*n + z*m with r,z,n from gi/gh psum, all [1, *]."""
                gib = vp.tile([1, R], F32, name=f"gib_{name}", tag="gib", bufs=2)
                nc.vector.tensor_tensor(gib[:], gi_ps[:], bi_sb[:], add)
                ghb = vp.tile([1, R], F32, name=f"ghb_{name}", tag="ghb", bufs=2)
                nc.vector.tensor_tensor(ghb[:], gh_ps[:], bh_sb[:], add)
                srz = vp.tile([1, 2 * S], F32, name=f"srz_{name}", tag="srz", bufs=2)
                nc.vector.tensor_tensor(
                    srz[:], gib[:, 0 : 2 * S], ghb[:, 0 : 2 * S], add
                )
                rz = vp.tile([1, 2 * S], F32, name=f"rz_{name}", tag="rz", bufs=2)
                nc.scalar.activation(rz[:], srz[:], SIG)
                t = vp.tile([1, S], F32, name=f"t_{name}", tag="t", bufs=2)
                nc.vector.tensor_tensor(t[:], rz[:, 0:S], ghb[:, 2 * S : R], mult)
                u = vp.tile([1, S], F32, name=f"u_{name}", tag="u", bufs=2)
                nc.vector.tensor_tensor(u[:], gib[:, 2 * S : R], t[:], add)
                n = vp.tile([1, S], F32, name=f"n_{name}", tag="n", bufs=2)
                nc.scalar.activation(n[:], u[:], TANH)
                d = vp.tile([1, S], F32, name=f"d_{name}", tag="d", bufs=2)
                nc.vector.tensor_tensor(d[:], mv_sb[:], n[:], sub)
                e = vp.tile([1, S], F32, name=f"e_{name}", tag="e", bufs=2)
                nc.vector.tensor_tensor(e[:], rz[:, S : 2 * S], d[:], mult)
                nc.vector.tensor_tensor(h_sb[:], n[:], e[:], add)

            # ---------- PE warm-up helper (keeps HAM at K=8/8) ----------
            warm_ps = pp.tile([1, 128], F32, name="warm_ps", tag="warm", bufs=1)

            def prewarm(tag, n):
                for i in range(n):
                    nc.tensor.matmul(
                        warm_ps[0:1, :],
                        lhsT=ones_sb[:, 0:1],
                        rhs=ones_sb[:, 0:128],
                        start=True,
                        stop=True,
                    )

            # ---------- layer 0 ----------
            prewarm("a", 24)
            gi0_ps = gates("gi0", KI0, d_wi0, p_sb, "gi", 1)
            gh0_ps = gates("gh0", KH, d_wh0, m0c_sb, "gh", 2)
            h0v_sb = sp.tile([1, S], F32, name="h0v_sb")
            gate_math("l0", gi0_ps, gh0_ps, bi0_sb, bh0_sb, m0v_sb, h0v_sb)

            # ---------- AllGather h0 (8 x 256 -> 2048), fp16 on the wire ----------
            ag_in = dp.tile([1, S], F16, name="ag_in")
            ag_out = dp.tile([KH, 128], F16, name="ag_out")
            nc.gpsimd.dma_start(ag_in[:], h0v_sb[:])  # f32 -> f16 cast (SWDGE)
            nc.gpsimd.collective_compute(
                "AllGather",
                mybir.AluOpType.bypass,
                replica_groups=[list(range(M))],
                ins=[ag_in[:, :].opt()],
                outs=[ag_out[:, :].opt()],
            )
            h0c_sb = sp.tile([128, KH], F16, name="h0c_sb")
            nc.gpsimd.dma_start(h0c_sb[:], ag_out[:, :].rearrange("j p -> p j"))

            # ---------- layer 1 (gh1 first: no dependence on h0) ----------
            gh1_ps = gates("gh1", KH, d_wh1, m1c_sb, "gh", 2)
            prewarm("b", 90)
            gi1_ps = gates("gi1", KH, d_wi1, h0c_sb, "gi", 1)
            h1v_sb = sp.tile([1, S], F32, name="h1v_sb")
            gate_math("l1", gi1_ps, gh1_ps, bi1_sb, bh1_sb, m1v_sb, h1v_sb)

            # ---------- merged/residual outputs ----------
            mrg_sb = sp.tile([1, S], F32, name="mrg_sb")
            nc.vector.tensor_tensor(mrg_sb[:], h1v_sb[:], res_sb[:], add)
            nres_sb = sp.tile([1, S], F32, name="nres_sb")
            nc.scalar.activation(nres_sb[:], mrg_sb[:], SIG)

            # ---------- output-projection partial: o = W_out[:, sl] @ merged ----------
            mb_ps = pp.tile([IO, S], F32, name="mb_ps", tag="xb", bufs=1)
            nc.tensor.matmul(
                mb_ps[:], lhsT=ones_sb[:, 0:IO], rhs=mrg_sb[:], start=True, stop=True
            )
            oprod = sp.tile([IO, S], F32, name="oprod")
            o_sb = sp.tile([IO, 1], F32, name="o_sb")
            nc.vector.tensor_tensor(oprod[:], wout_sb[:], mb_ps[:], mult)
            nc.vector.tensor_reduce(o_sb[:], oprod[:], mybir.AxisListType.X, add)

            # ---------- outputs (HWDGE, end of stream) ----------
            dma(o_h0[:, :], h0v_sb[:])
            dma(o_h1[:, :], h1v_sb[:])
            dma(o_nres[:, :], nres_sb[:])
            dma(o_op[:, :], o_sb[:])

    nc.compile()
    return nc


def _shards(inputs):
    """Build the 8 per-core input maps from the full-size numpy inputs."""
    x = np.asarray(inputs["x"], np.float32)
    residual = np.asarray(inputs["residual"], np.float32)
    memory = np.asarray(inputs["memory"], np.float32)
    W_proc = np.asarray(inputs["W_proc"], np.float32)
    b_proc = np.asarray(inputs["b_proc"], np.float32)
    W_out = np.asarray(inputs["W_out"], np.float32)

    # shared (replicated) tensors
    x_row = np.ascontiguousarray(x.reshape(1, IO))
    wproc_r = np.ascontiguousarray(W_proc.reshape(KI0, 128, IO).transpose(1, 0, 2))
    bproc_c = np.ascontiguousarray(b_proc.reshape(KI0, 128).T)
    m0 = memory[0, 0]
    m1 = memory[1, 0]
    m0_col = np.ascontiguousarray(m0.reshape(KH, 128).T.astype(np.float16))
    m1_col = np.ascontiguousarray(m1.reshape(KH, 128).T.astype(np.float16))

    def wt(w, rows, nk):
        w16 = np.asarray(w, np.float32)[rows].T.astype(np.float16)
        return np.ascontiguousarray(w16).reshape(nk // GB, GB, 128, R)

    maps = []
    for c in range(M):
        sl = np.arange(S * c, S * (c + 1))
        rows = np.concatenate([sl, MEM + sl, 2 * MEM + sl])
        mp = {
            "x_row": x_row,
            "wproc_r": wproc_r,
            "bproc_c": bproc_c,
            "m0_col": m0_col,
            "m1_col": m1_col,
            "m0_vec": np.ascontiguousarray(m0[sl].reshape(1, S)),
            "m1_vec": np.ascontiguousarray(m1[sl].reshape(1, S)),
            "res_vec": np.ascontiguousarray(residual[0, 0, sl].reshape(1, S)),
            "bi0": np.ascontiguousarray(inputs["b_ih0"][rows].reshape(1, R)).astype(np.float32),
            "bh0": np.ascontiguousarray(inputs["b_hh0"][rows].reshape(1, R)).astype(np.float32),
            "bi1": np.ascontiguousarray(inputs["b_ih1"][rows].reshape(1, R)).astype(np.float32),
            "bh1": np.ascontiguousarray(inputs["b_hh1"][rows].reshape(1, R)).astype(np.float32),
            "wih0t": wt(inputs["w_ih0"], rows, KI0),
            "whh0t": wt(inputs["w_hh0"], rows, KH),
            "whh1t": wt(inputs["w_hh1"], rows, KH),
            "wih1t": wt(inputs["w_ih1"], rows, KH),
            "woutn": np.ascontiguousarray(W_out[:, sl]),
        }
        maps.append(mp)
    return maps


def _run(inputs, **kwargs):
    if "nc" not in _CACHE:
        _CACHE["nc"] = _build()
    nc = _CACHE["nc"]
    in_maps = _shards(inputs)
    res = run_bass_kernel_spmd(nc, in_maps, core_ids=list(range(M)), **kwargs)
    return res


def _assemble(results, inputs):
    b_out = np.asarray(inputs["b_out"], np.float32)
    h0 = np.concatenate([results[c]["h0_out"].ravel() for c in range(M)])
    h1 = np.concatenate([results[c]["h1_out"].ravel() for c in range(M)])
    nres = np.concatenate([results[c]["nres_out"].ravel() for c in range(M)])
    opart = np.sum([results[c]["opart_out"].ravel() for c in range(M)], axis=0)
    output = (opart + b_out).astype(np.float32).reshape(1, 1, IO)
    new_residual = nres.astype(np.float32).reshape(1, 1, MEM)
    new_memory = np.stack([h0, h1]).astype(np.float32).reshape(2, 1, MEM)
    return output, new_residual, new_memory


def kernel(**inputs):
    res = _run(inputs)
    return _assemble(res.results, inputs)
